# revision 3
# baseline (speedup 1.0000x reference)
"""Trainium2 Bass kernel for nn_ContextProjector (moe_routing).

Reference computation:
    projected = split_heads(x @ W_x + b_x)            # (B,H,N,D)
    fx        = split_heads(x @ W_fx + b_fx)          # (B,H,N,D)
    sp        = projected @ W_slice + b_slice         # (B,H,N,S)
    w         = softmax(sp / clip(temp,.5,5))         # (B,H,N,S)
    norm      = w.sum(axis=N)                         # (B,H,S)
    out       = einsum('bhns,bhnd->bhsd', w/(norm+.01), fx)

Key algebraic restructuring (all exact):
  * projected is only used for sp, so fold on host:
        Wc[c,(h,s)] = sum_d W_x[c,(h,d)] W_slice[d,s] / t[h]
    and sp/t = x @ Wc + bc.  The additive bias bc is applied
    MULTIPLICATIVELY after exp:  exp(lg + bc) = exp(lg) * E,  E = exp(bc),
    which moves it off the TensorE (saves one 512-row matmul per subtile,
    20% of PE time) onto a coalesced 2x-mode DVE multiply.
  * fx never exists on device. With w~ the per-token softmax:
        sum_n w~[n,s] (x[n,:] @ W_fx + b_fx)[d]
          = (sum_n w~[n,s] [x[n,:] | 1]) @ [W_fx; b_fx]
    so the device only accumulates G[(h,s), c] = sum_n w~[n,(h,s)] [x|1][n,c]
    into PSUM; the tiny G @ W_fx, the b_fx term, and the final divide by
    (norm+0.01) happen on host in float64. Column c=C of G is the norm.

Device per core (8 cores: core = 4*b + quarter-of-N, 16384 tokens each).
Work unit is a QUAD (4 subtiles = 512 tokens) to amortize per-instruction
fixed costs on DVE/GpSimd:
  PE : per pair: lg2 psum[128,2,512] = 4 matmuls (2 K-chunks x 2 subtiles)
  ACT: u4[:, 2p:2p+2, :] = exp(lg2) -> fp16 SBUF   (2 instrs per quad)
  DVE: w4 = u4 * E         (coalesced 2048-elem multiply, 2x mode)
  DVE: v1 = w[s<32]+w[s>=32]; v2 = v1[s<16]+v1[s>=16]   (packed-run adds)
  --- next quad (software skew) ---
  DVE: den = reduce_s(v2); rec = 1/den
  DVE: w[h=0]  *= rec[h=0]        (head-split normalize)
  GpS: w[h>=1] *= rec[h>=1]
  PE : per head-pair j: G_psum[j] += w[:, chunk]^T @ [x|1]  (4 MMs, N=257)
G matmuls are emitted PD quads behind their producers (software pipelining)
so the PE never stalls on the softmax chain; 12 warm-up matmuls at kernel
start hold the PE clock-gate up through the first DMA. All matmul operands
fp16 (PSUM accumulates fp32).
"""

import numpy as np

import concourse.bass as bass
import concourse.mybir as mybir
import concourse.tile as tile
from concourse import bacc
from concourse.bass_utils import run_bass_kernel_spmd

# Problem shape (hardcoded per contract)
B, N, C = 2, 65536, 256
H, D, S = 8, 64, 64
HS = H * S    # 512
P = 128
NCORES = 8
SHARDS_PER_B = NCORES // B   # 4
T = N // SHARDS_PER_B        # 16384 tokens per core
CA = C + 1                   # token-major x augmented with a ones column

HD = 1     # normalize head-split: h<HD on DVE, h>=HD on GpSimd
PD = 2     # G-matmul pipeline depth (quads)
TT = 2048  # tokens per DMA block
QT = 512   # tokens per quad

f16 = mybir.dt.float16
f32 = mybir.dt.float32


def _emit(ctx, tc, xt, wc, e4, xtm, out, t_tokens):
    nc = tc.nc
    KO = C // P              # 2 K-chunks of x
    n_blk = t_tokens // TT
    n_sub = TT // P          # subtiles (128 tokens) per block
    n_tot = t_tokens // P    # total subtiles (for G start/stop flags)
    QS = QT // P             # 4 subtiles per quad

    consts = ctx.enter_context(tc.tile_pool(name="consts", bufs=1))
    xpool = ctx.enter_context(tc.tile_pool(name="xpool", bufs=3))
    mpool = ctx.enter_context(tc.tile_pool(name="mpool", bufs=3))
    upool = ctx.enter_context(tc.tile_pool(name="upool", bufs=2))
    wpool = ctx.enter_context(tc.tile_pool(name="wpool", bufs=PD + 3))
    vpool = ctx.enter_context(tc.tile_pool(name="vpool", bufs=2))
    spool = ctx.enter_context(tc.tile_pool(name="spool", bufs=3))
    ppool = ctx.enter_context(tc.tile_pool(name="ppool", bufs=2, space="PSUM"))
    apool = ctx.enter_context(tc.tile_pool(name="apool", bufs=1, space="PSUM"))
    opool = ctx.enter_context(tc.tile_pool(name="opool", bufs=1))

    # Constant weights, resident in SBUF for the whole kernel.
    wc_sb = consts.tile([P, KO, HS], f16)
    nc.sync.dma_start(wc_sb[:], wc[:].rearrange("(ko ki) n -> ki ko n", ki=P))
    e4_sb = consts.tile([P, QS, HS], f16)
    nc.sync.dma_start(e4_sb[:], e4[:].rearrange("p (t n) -> p t n", t=QS))

    # Persistent PSUM accumulators: head-pair j holds
    # G[(2 heads x 64 s), 257] = sum_n w~[n, (h,s)] * [x[n, :] | 1].
    accs = [apool.tile([P, CA], f32, tag=f"acc{j}", name=f"acc{j}")
            for j in range(4)]

    xt_r = xt[:].rearrange("(ko ki) t -> ki ko t", ki=P)

    # HAM warm-up: keep the PE busy during the initial DMAs so the clock
    # gate ramps before real work starts.
    wup = consts.tile([P, HS], f16)
    nc.gpsimd.memset(wup[:], 0.0)
    for _ in range(12):
        warm = ppool.tile([P, 2, HS], f32, tag="lg", name="warm")
        nc.tensor.matmul(warm[:, 0, :], wup[:, 0:P], wup[:], start=True,
                         stop=True)

    def emit_g(w4, xm_sb, quad, gi0):
        # reduction matmuls for a normalized quad (delayed PD quads so PE
        # always has normalized weights available)
        for pi in range(QS):
            gi = gi0 + pi
            rhs = xm_sb[:, quad * QS + pi, :]            # [128(tok), 257]
            for j in range(4):
                lhsT = w4[:, pi, j * P:(j + 1) * P]      # [128(tok), 128]
                nc.tensor.matmul(accs[j][:], lhsT, rhs,
                                 start=gi == 0, stop=gi == n_tot - 1)

    def chain2(e):
        # second half of the softmax chain for a quad (one quad behind its
        # producer so no engine head-blocks on a cross-engine dep)
        w4, v2 = e
        den = spool.tile([P, QS, H], f32, tag="den")
        nc.vector.tensor_reduce(out=den[:], in_=v2[:],
                                axis=mybir.AxisListType.X,
                                op=mybir.AluOpType.add)
        rec = spool.tile([P, QS, H], f16, tag="rec")
        with nc.allow_low_precision(reason="softmax denom reciprocal in f16"):
            nc.vector.reciprocal(rec[:], den[:])
        wv = w4[:].rearrange("p t (h s) -> p t h s", h=H)
        nc.vector.tensor_mul(
            out=wv[:, :, 0:HD, :], in0=wv[:, :, 0:HD, :],
            in1=rec[:, :, 0:HD, None].to_broadcast((P, QS, HD, S)))
        nc.gpsimd.tensor_mul(
            out=wv[:, :, HD:H, :], in0=wv[:, :, HD:H, :],
            in1=rec[:, :, HD:H, None].to_broadcast((P, QS, H - HD, S)))

    chain = []
    pending = []
    qr = 0
    for blk in range(n_blk):
        x_sb = xpool.tile([P, KO, TT], f16)
        xm_sb = mpool.tile([P, n_sub, CA], f16)
        xm_src = xtm[blk * TT:(blk + 1) * TT, :].rearrange(
            "(sb p) c -> p sb c", p=P)
        if blk == 0:
            # split the first block's DMAs so compute can start after the
            # first quarter arrives
            for i in range(4):
                sl = slice(i * TT // 4, (i + 1) * TT // 4)
                nc.sync.dma_start(x_sb[:, :, sl], xt_r[:, :, sl])
            nc.sync.dma_start(xm_sb[:, 0:n_sub // 2, :],
                              xm_src[:, 0:n_sub // 2, :])
            nc.sync.dma_start(xm_sb[:, n_sub // 2:, :],
                              xm_src[:, n_sub // 2:, :])
        else:
            nc.sync.dma_start(x_sb[:], xt_r[:, :, blk * TT:(blk + 1) * TT])
            nc.sync.dma_start(xm_sb[:], xm_src)
        for quad in range(n_sub // QS):
            if chain:
                chain2(chain.pop(0))
            u4 = upool.tile([P, QS, HS], f16)
            for half in range(2):
                lg2 = ppool.tile([P, 2, HS], f32, tag="lg")
                for si in range(2):
                    sub = quad * QS + half * 2 + si
                    xk0 = x_sb[:, 0, sub * P:(sub + 1) * P]
                    xk1 = x_sb[:, 1, sub * P:(sub + 1) * P]
                    nc.tensor.matmul(lg2[:, si, :], xk0, wc_sb[:, 0],
                                     start=True, stop=False)
                    nc.tensor.matmul(lg2[:, si, :], xk1, wc_sb[:, 1],
                                     start=False, stop=True)
                nc.scalar.activation(out=u4[:, half * 2:half * 2 + 2, :],
                                     in_=lg2[:],
                                     func=mybir.ActivationFunctionType.Exp)
            w4 = wpool.tile([P, QS, HS], f16)
            nc.vector.tensor_mul(out=w4[:], in0=u4[:], in1=e4_sb[:])
            wv = w4[:].rearrange("p t (h s) -> p t h s", h=H)
            v1 = vpool.tile([P, QS, H, 32], f16, tag="v1")
            nc.vector.tensor_add(out=v1[:], in0=wv[:, :, :, 0:32],
                                 in1=wv[:, :, :, 32:64])
            v2 = vpool.tile([P, QS, H, 16], f16, tag="v2")
            nc.vector.tensor_add(out=v2[:], in0=v1[:, :, :, 0:16],
                                 in1=v1[:, :, :, 16:32])
            chain.append((w4, v2))
            pending.append((w4, xm_sb, quad, qr * QS))
            if len(pending) > PD:
                emit_g(*pending.pop(0))
            qr += 1
    while chain:
        chain2(chain.pop(0))
    while pending:
        emit_g(*pending.pop(0))

    # spread the final PSUM evictions across engines so they don't
    # serialize behind DVE's per-op DRAIN at the kernel tail
    out_sb = opool.tile([P, 4, CA], f32)
    for j in range(4):
        if j % 2 == 0:
            nc.vector.tensor_copy(out_sb[:, j, :], accs[j][:])
        else:
            nc.scalar.activation(out=out_sb[:, j, :], in_=accs[j][:],
                                 func=mybir.ActivationFunctionType.Copy)
    nc.sync.dma_start(out[:].rearrange("j p c -> p j c"), out_sb[:])


def build_bass(t_tokens=T, finalize=True):
    from contextlib import ExitStack
    nc = bacc.Bacc("TRN2")
    xt = nc.dram_tensor("xt", [C, t_tokens], f16, kind="ExternalInput")
    wc = nc.dram_tensor("wc", [C, HS], f16, kind="ExternalInput")
    e4 = nc.dram_tensor("e4", [P, 4 * HS], f16, kind="ExternalInput")
    xtm = nc.dram_tensor("xtm", [t_tokens, CA], f16, kind="ExternalInput")
    out = nc.dram_tensor("out", [4, P, CA], f32, kind="ExternalOutput")
    with tile.TileContext(nc) as tc:
        with ExitStack() as ctx:
            _emit(ctx, tc, xt, wc, e4, xtm, out, t_tokens)
    if finalize:
        nc.finalize()
    return nc


def make_device_weights(W_x, b_x, W_slice, b_slice, temperature):
    """Host-side weight fusion -> wc_dev [C, HS] f16 (h-major cols) and
    e4_dev [128, 4*HS] f16 = exp(bc) replicated."""
    temp = np.clip(np.asarray(temperature, np.float64).reshape(H), 0.5, 5.0)
    Wx3 = np.asarray(W_x, np.float64).reshape(C, H, D)
    Ws = np.asarray(W_slice, np.float64)
    Wc = np.einsum("chd,ds->chs", Wx3, Ws) / temp[None, :, None]
    bc = (np.asarray(b_x, np.float64).reshape(H, D) @ Ws
          + np.asarray(b_slice, np.float64)[None, :]) / temp[:, None]
    wc_dev = Wc.reshape(C, HS).astype(np.float16)
    e_row = np.exp(bc.reshape(HS))                       # [HS] h-major
    e4_dev = np.broadcast_to(
        np.tile(e_row, 4).astype(np.float16), (P, 4 * HS)).copy()
    return wc_dev, e4_dev


def untangle(M):
    """Per-core device output [4, 128, 257] -> G [H, S, C+1] (col C = norm)."""
    M = np.asarray(M, np.float64)
    G = np.empty((H, S, CA), np.float64)
    for j in range(4):
        G[2 * j] = M[j, 0:S, :]
        G[2 * j + 1] = M[j, S:2 * S, :]
    return G


def postprocess(core_outs, W_fx, b_fx):
    Wf = np.asarray(W_fx, np.float64).reshape(C, H, D)
    bfx = np.asarray(b_fx, np.float64).reshape(H, D)
    out = np.empty((B, H, S, D), np.float32)
    for b in range(B):
        G = sum(untangle(core_outs[b * SHARDS_PER_B + q]) for q in range(SHARDS_PER_B))
        Mn = G[..., C]                      # [H, S] total softmax mass
        Q = np.einsum("hsc,chd->hsd", G[..., :C], Wf)
        res = (Q + Mn[..., None] * bfx[:, None, :]) / (Mn[..., None] + 0.01)
        out[b] = res.astype(np.float32)
    return out


def make_in_maps(x, wc_dev, e4_dev):
    x = np.asarray(x)
    in_maps = []
    for core in range(NCORES):
        b, q = core // SHARDS_PER_B, core % SHARDS_PER_B
        xs = x[b, q * T:(q + 1) * T, :]
        xt = np.ascontiguousarray(xs.T.astype(np.float16))
        xtm = np.empty((T, CA), np.float16)
        xtm[:, :C] = xs.astype(np.float16)
        xtm[:, C] = 1.0
        in_maps.append({"xt": xt, "wc": wc_dev, "e4": e4_dev, "xtm": xtm})
    return in_maps


_NC_CACHE = {}


def _get_nc():
    if "nc" not in _NC_CACHE:
        _NC_CACHE["nc"] = build_bass()
    return _NC_CACHE["nc"]


def _run(x, W_x, b_x, W_fx, b_fx, W_slice, b_slice, temperature, trace=False):
    wc_dev, e4_dev = make_device_weights(W_x, b_x, W_slice, b_slice, temperature)
    in_maps = make_in_maps(x, wc_dev, e4_dev)
    res = run_bass_kernel_spmd(_get_nc(), in_maps, core_ids=list(range(NCORES)),
                               trace=trace)
    out = postprocess([r["out"] for r in res.results], W_fx, b_fx)
    return out, res


def kernel(**inputs) -> np.ndarray:
    out, _ = _run(**inputs)
    return out


def kernel_traced(**inputs):
    out, res = _run(**inputs, trace=True)
    return out, res
